# revision 4
# baseline (speedup 1.0000x reference)
"""Sparse Adagrad (Habana-style) on 8 Trainium2 NeuronCores.

Row-shard the tables across 8 cores by index range (62500 rows each).
Only the TOUCHED rows (~20.6k per core, ~33%) are shipped to the
device, compacted into a [128 partitions x RP blocks] layout chosen by
the host; untouched rows pass through on the host. All device traffic
is fp16 (tolerance is 2e-2; we land ~1e-3).

Compact layout: touched rows are sorted by duplicate-count (desc) and
snake-dealt across RP blocks; the i-th dealt row lands at block
j = snake(i % RP), height h = i // RP, i.e. table position
(partition h, column j). Each row's FIRST gradient occurrence is
stored at base slot h of block j, so the base scatter matrix is the
IDENTITY (constant). Duplicate occurrences (~4.4k/core) are pooled per
4-block PSUM-bank group (up to 128 slots) with a one-hot A_ovf built
on device via is_equal against an iota.

Per PSUM bank (4 blocks, psum[:, 4b:4b+4, 0:128] with [Sum g | Sum g2]
halves per block):
    bank = I @ gsq[4 blocks]            (identity matmul, start=True)
    bank += A_ovf[grp] @ go_diag[grp]   (block-diagonal overflow rhs,
                                         stop=True, same footprint —
                                         HW requires accumulation
                                         groups to open/close with
                                         identical out regions)
then
    m'  = m + Sum g2                          (DVE; GPSIMD can't read PSUM)
    r   = AbsRsqrt(m'*(1/lr^2) + eps)         (ACT)  [= lr*rsqrt(m')]
    u   = r * Sum g                           (DVE)  [= lr*Sum g/sqrt(m')]
outputs [u | m'] per row; the host applies w' = w - u in f32 during
assembly (w never round-trips through fp16). The denominator uses the
fully accumulated m' and is constant across duplicates, so it factors
out of the sum — matching the reference exactly.
"""

import sys

for _p in ("/opt/trn_rl_repo", "/root/.axon_site/_ro/trn_rl_repo"):
    if _p not in sys.path:
        sys.path.insert(0, _p)

import numpy as np

P = 128          # SBUF partitions
D = 64           # embedding dim
NCORES = 8
VC = 62500       # table rows per core
OVF = 32         # overflow slots per block
JSUB = 8   # blocks per compute step (PSUM tile = 2 banks -> 4 bufs)

_program_cache = {}


def _build_program(rp, reps=1, rsqrt='act'):
    """rp: number of blocks (table columns) per core; rp % 12 == 0, % 4 == 0."""
    from concourse import bacc, mybir
    import concourse.tile as tile

    nit = rp // JSUB
    assert nit * JSUB == rp and rp % 4 == 0
    rp4 = rp // 4
    f32 = mybir.dt.float32
    f16 = mybir.dt.float16
    nc = bacc.Bacc("TRN2", target_bir_lowering=False, debug=False,
                   num_devices=NCORES)

    # [m | g] interleaved per block (single input stream); host applies
    # w' = w - u during assembly
    mg_in = nc.dram_tensor("mg_in", [P, rp * 2 * D], f16,
                           kind="ExternalInput")
    go_in = nc.dram_tensor("go_in", [P, rp4 * 4 * 2 * D], f16,
                          kind="ExternalInput")
    midxo = nc.dram_tensor("midxo", [P, rp4], f16, kind="ExternalInput")
    scal = nc.dram_tensor("scal", [1, 2], f32, kind="ExternalInput")  # [inv_lr2, eps]
    # output: [u | m'] interleaved per block
    um_out = nc.dram_tensor("um_out", [P, rp * 2 * D], f16,
                            kind="ExternalOutput")

    with tile.TileContext(nc) as tc:
        with tc.tile_pool(name="consts", bufs=1) as consts, \
             tc.tile_pool(name="big", bufs=3) as bigpool, \
             tc.tile_pool(name="sbuf", bufs=4) as pool, \
             tc.tile_pool(name="psum", bufs=4, space="PSUM") as psum:
            # iota along free dim (same in every partition), fp16
            iota_i = consts.tile([P, P], mybir.dt.int32)
            nc.gpsimd.iota(iota_i[:], pattern=[[1, P]], base=0,
                           channel_multiplier=0)
            iota_f = consts.tile([P, P], f16)
            nc.vector.tensor_copy(iota_f[:], iota_i[:])
            # partition index (one value per partition), fp16
            piota_i = consts.tile([P, 1], mybir.dt.int32)
            nc.gpsimd.iota(piota_i[:], pattern=[[1, 1]], base=0,
                           channel_multiplier=1)
            piota_f = consts.tile([P, 1], f16)
            nc.vector.tensor_copy(piota_f[:], piota_i[:])
            # identity matrix [p, f] = (f == p), fp16
            ident = consts.tile([P, P], f16)
            nc.vector.tensor_tensor(
                out=ident[:],
                in0=iota_f[:],
                in1=piota_f[:].to_broadcast((P, P)),
                op=mybir.AluOpType.is_equal,
            )

            inv_lr2 = consts.tile([P, 1], f32)
            nc.sync.dma_start(out=inv_lr2[:],
                              in_=scal[:, 0:1].to_broadcast((P, 1)))
            eps_t = consts.tile([P, 1], f32)
            nc.sync.dma_start(out=eps_t[:],
                              in_=scal[:, 1:2].to_broadcast((P, 1)))

            # overflow gradients, block-diagonal per 4-block group:
            # go_s[slot, grp, db, 0:64]=g, [64:128]=g^2 of that slot if it
            # belongs to sub-block db, else zero. Resident all sweep.
            go_s = consts.tile([P, rp4, 4, 2 * D], f16)
            nc.sync.dma_start(out=go_s[:], in_=go_in[:])
            midxo_s = consts.tile([P, rp4], f16)
            nc.sync.dma_start(out=midxo_s[:], in_=midxo[:])

            # A_ovf[slot, grp, p] = (midxo[slot, grp] == p)
            a_ovf = consts.tile([P, rp4, P], f16)
            nc.vector.tensor_tensor(
                out=a_ovf[:],
                in0=midxo_s[:, :, None].broadcast_to((P, rp4, P)),
                in1=iota_f[:, None, :].broadcast_to((P, rp4, P)),
                op=mybir.AluOpType.is_equal,
            )

            import contextlib

            def _rep_scope():
                return contextlib.nullcontext()

            with _rep_scope():
              for _rep in range(reps):
                NH = 3
                for it2 in range(nit // NH):
                    # DMA at 2-iteration granularity — bigger transfers
                    # amortize DGE latency while staying fine-grained enough
                    # to overlap with compute; compute stays at JSUB blocks
                    # per step (PSUM size).
                    J2 = NH * JSUB
                    j00 = it2 * J2
                    mg2 = bigpool.tile([P, NH, JSUB, 2, D], f16)
                    nc.sync.dma_start(
                        out=mg2[:],
                        in_=mg_in[:, j00 * 2 * D:(j00 + J2) * 2 * D])
                    um2_n = bigpool.tile([P, NH, JSUB, 2 * D], f16)
                    for half in range(NH):
                        j0 = j00 + half * JSUB

                        # [g | g^2] rhs tile: ACT fills both halves
                        gsq = pool.tile([P, JSUB, 2 * D], f16)
                        gb_v = mg2[:, half, :, 1, :]
                        nc.scalar.copy(gsq[:, :, 0:D], gb_v)
                        nc.gpsimd.tensor_tensor(
                            out=gsq[:, :, D:2 * D], in0=gb_v, in1=gb_v,
                            op=mybir.AluOpType.mult)

                        ps = psum.tile([P, JSUB, 2 * D], f32)
                        # Per-region accumulation groups (open and close with
                        # the SAME out footprint — HW/NEFF rejects mismatched
                        # group shapes). start=True lazily marks the whole
                        # 2KB bank pending-zero, so each region's overflow
                        # accumulate must land before the next start touches
                        # that bank: waves of 3 regions in 3 distinct banks
                        # {w, w+4, w+8}, which also share the identity
                        # stationary across 3 matmuls (alternating weights
                        # cost ~3x on PE).
                        # HW requires accumulation groups to open and
                        # close with the SAME out footprint: both the base
                        # (identity) and overflow matmuls cover one whole
                        # PSUM bank (4 blocks, N=512). The overflow rhs is
                        # block-diagonal so one 128-slot matmul serves the
                        # bank's 4 blocks.
                        for b in range(JSUB // 4):
                            nc.tensor.matmul(
                                out=ps[:, 4 * b:4 * (b + 1), :],
                                lhsT=ident[:],
                                rhs=gsq[:, 4 * b:4 * (b + 1), :],
                                start=True, stop=False,
                                skip_group_check=True,
                            )
                        for b in range(JSUB // 4):
                            grp = j0 // 4 + b
                            nc.tensor.matmul(
                                out=ps[:, 4 * b:4 * (b + 1), :],
                                lhsT=a_ovf[:, grp, :],
                                rhs=go_s[:, grp, :, :],
                                start=False, stop=True,
                                skip_group_check=True,
                            )

                        # m' = m + Sum g^2  (psum high half; GPSIMD cannot
                        # read PSUM, so this lives on DVE)
                        nc.vector.tensor_tensor(
                            out=um2_n[:, half, :, D:2 * D],
                            in0=ps[:, :, D:2 * D],
                            in1=mg2[:, half, :, 0, :],
                            op=mybir.AluOpType.add,
                        )
                        # r = 1/sqrt(m'*inv_lr2 + eps) [= lr*rsqrt(m')]
                        # in one ACT op; input >= 0 so the |x| is a no-op.
                        # (CoreSim lacks the fused op: rsqrt='split' swaps in
                        # the equivalent Sqrt + reciprocal chain for sim.)
                        r_t = pool.tile([P, JSUB, D], f32)
                        if rsqrt == 'act':
                            nc.scalar.activation(
                                r_t[:], um2_n[:, half, :, D:2 * D],
                                mybir.ActivationFunctionType.
                                Abs_reciprocal_sqrt,
                                bias=eps_t[:], scale=inv_lr2[:])
                        else:
                            s_t = pool.tile([P, JSUB, D], f32)
                            nc.scalar.activation(
                                s_t[:], um2_n[:, half, :, D:2 * D],
                                mybir.ActivationFunctionType.Sqrt,
                                bias=eps_t[:], scale=inv_lr2[:])
                            nc.vector.reciprocal_approx_fast(out=r_t[:],
                                                             in_=s_t[:])
                        # u = r * Sum g   [= lr * Sum g / sqrt(m')]
                        nc.vector.tensor_tensor(
                            out=um2_n[:, half, :, 0:D],
                            in0=ps[:, :, 0:D],
                            in1=r_t[:],
                            op=mybir.AluOpType.mult,
                        )
                    nc.scalar.dma_start(
                        out=um_out[:, j00 * 2 * D:(j00 + J2) * 2 * D],
                        in_=um2_n[:])

    nc.compile()
    return nc


def get_program(rp, **opts):
    key = (rp, tuple(sorted(opts.items())))
    if key not in _program_cache:
        _program_cache[key] = _build_program(rp, **opts)
    return _program_cache[key]


def _choose_rp(max_touched):
    # rp must be a multiple of 12 (JSUB) and 4; 12 covers both.
    rp = -(-max_touched // P)
    rp = -(-rp // 12) * 12
    return rp


def prepare_inputs(gradients, weights, moments, indices, learning_rate,
                   valid_count):
    """Host routing: find touched rows per core, snake-deal them into a
    compact [128, rp] table layout, place first occurrences at identity
    slots and duplicates into per-block overflow chunks."""
    g = np.asarray(gradients, dtype=np.float32)
    w = np.asarray(weights, dtype=np.float32)
    m = np.asarray(moments, dtype=np.float32)
    idx = np.asarray(indices).astype(np.int64)
    vc = int(valid_count)
    lr = float(np.asarray(learning_rate, dtype=np.float32).reshape(-1)[0])

    idxv = idx[:vc]
    gv = g[:vc]
    owner = idxv // VC
    loc = idxv - owner * VC

    per_core = []
    max_touched = 0
    for c in range(NCORES):
        mask = owner == c
        idc = loc[mask]
        gc = gv[mask]
        rows, inv, counts = np.unique(idc, return_inverse=True,
                                      return_counts=True)
        per_core.append((idc, gc, rows, inv, counts))
        max_touched = max(max_touched, len(rows))

    rp = _choose_rp(max_touched)
    # retry with larger rp if overflow slots per block exceed OVF
    for attempt in range(6):
        ok = True
        packed = []
        for c in range(NCORES):
            pc = _pack_core(per_core[c], rp)
            if pc is None:
                ok = False
                break
            packed.append(pc)
        if ok:
            break
        rp += 12
    else:
        return None

    inv_lr2 = 1.0 / (lr * lr)
    eps = 1e-12
    scal = np.array([[inv_lr2, eps]], dtype=np.float32)

    in_maps = []
    unpack_info = []
    for c in range(NCORES):
        h_of, j_of, rows, gb, go, midxo = packed[c]
        mgdev = np.zeros((P, rp, 2, D), dtype=np.float16)
        base = c * VC
        mgdev[h_of, j_of, 0] = m[base + rows].astype(np.float16)
        mgdev[:, :, 1, :] = gb
        in_maps.append({
            "mg_in": mgdev.reshape(P, rp * 2 * D),
            "go_in": go.reshape(P, (rp // 4) * 4 * 2 * D),
            "midxo": midxo,
            "scal": scal,
        })
        unpack_info.append((h_of, j_of, rows))
    return in_maps, rp, unpack_info


def _pack_core(pc, rp):
    """Snake-deal rows into rp blocks; returns (h, j, rows, g_base, g_ovf,
    midx_ovf) or None if an overflow chunk exceeds OVF slots."""
    idc, gc, rows, inv, counts = pc
    T = len(rows)
    if T > P * rp:
        return None
    # deal rows sorted by dup count (desc) so block weights balance
    order = np.argsort(-counts, kind="stable")
    pos = np.arange(T, dtype=np.int64)
    rounds = pos // rp
    k = pos % rp
    j_sorted = np.where(rounds % 2 == 0, k, rp - 1 - k)
    h_sorted = rounds
    # h_of[i], j_of[i] = placement of rows[order[i]] -> map back to row order
    h_of = np.empty(T, dtype=np.int64)
    j_of = np.empty(T, dtype=np.int64)
    h_of[order] = h_sorted
    j_of[order] = j_sorted

    assert rp % 4 == 0
    # occurrences: rank within row (stable sort by row id)
    n = len(idc)
    o = np.argsort(inv, kind="stable")
    starts = np.concatenate(([0], np.cumsum(counts)[:-1]))
    rank = np.empty(n, dtype=np.int64)
    rank[o] = np.arange(n, dtype=np.int64) - starts[inv[o]]

    occ_h = h_of[inv]
    occ_j = j_of[inv]

    g16 = gc.astype(np.float16)
    gb = np.zeros((P, rp, D), dtype=np.float16)
    first = rank == 0
    gb[occ_h[first], occ_j[first]] = g16[first]

    dup = ~first
    dj = occ_j[dup]
    dh = occ_h[dup]
    dg = g16[dup]
    # overflow slots are pooled per 4-block group (128 slots per group)
    dgrp = dj // 4
    db = dj % 4
    do = np.argsort(dgrp, kind="stable")
    gc_ = np.bincount(dgrp, minlength=rp // 4)
    if gc_.max() > P:
        return None
    gstarts = np.concatenate(([0], np.cumsum(gc_)[:-1]))
    slot = np.empty(len(dj), dtype=np.int64)
    slot[do] = np.arange(len(dj), dtype=np.int64) - gstarts[dgrp[do]]

    go = np.zeros((P, rp // 4, 4, 2 * D), dtype=np.float16)
    midxo = np.full((P, rp // 4), -1.0, dtype=np.float16)
    go[slot, dgrp, db, 0:D] = dg
    go[slot, dgrp, db, D:2 * D] = (dg.astype(np.float32) ** 2
                                   ).astype(np.float16)
    midxo[slot, dgrp] = dh.astype(np.float16)
    return h_of, j_of, rows, gb, go, midxo


def assemble_outputs(results, weights, moments, rp, unpack_info):
    w_new = np.array(weights, dtype=np.float32, copy=True)
    m_new = np.array(moments, dtype=np.float32, copy=True)
    for c in range(NCORES):
        h_of, j_of, rows = unpack_info[c]
        um = results[c]["um_out"].reshape(P, rp, 2 * D)
        base = c * VC
        w_new[base + rows] -= um[h_of, j_of, 0:D].astype(np.float32)
        m_new[base + rows] = um[h_of, j_of, D:2 * D].astype(np.float32)
    return w_new, m_new


def _host_reference(gradients, weights, moments, indices, lr, valid_count):
    g = np.asarray(gradients, dtype=np.float64).copy()
    g[int(valid_count):] = 0.0
    idx = np.asarray(indices).astype(np.int64)
    m_new = np.asarray(moments, dtype=np.float64).copy()
    np.add.at(m_new, idx, g * g)
    denom = np.sqrt(m_new[idx]) + 1e-10
    w_new = np.asarray(weights, dtype=np.float64).copy()
    np.add.at(w_new, idx, -lr * g / denom)
    return w_new.astype(np.float32), m_new.astype(np.float32)


def kernel(gradients, weights, moments, indices, learning_rate, valid_count):
    from concourse.bass_utils import run_bass_kernel_spmd

    lr = float(np.asarray(learning_rate, dtype=np.float32).reshape(-1)[0])
    if lr == 0.0:
        # Degenerate: weights unchanged, moments still accumulate g^2.
        g = np.asarray(gradients, dtype=np.float32).copy()
        g[int(valid_count):] = 0.0
        idx = np.asarray(indices).astype(np.int64)
        m_new = np.asarray(moments, dtype=np.float32).copy()
        np.add.at(m_new, idx, g * g)
        return np.asarray(weights, dtype=np.float32).copy(), m_new

    prep = prepare_inputs(
        gradients, weights, moments, indices, learning_rate, valid_count)
    if prep is None:
        # Pathological duplicate distribution the packer can't place
        # (not reachable for uniform indices): host fallback.
        return _host_reference(gradients, weights, moments, indices,
                               lr, valid_count)
    in_maps, rp, unpack_info = prep
    nc = get_program(rp)
    res = run_bass_kernel_spmd(nc, in_maps, core_ids=list(range(NCORES)))
    return assemble_outputs(res.results, weights, moments, rp, unpack_info)
